# revision 4
# baseline (speedup 1.0000x reference)
"""GCN layer (gather + segment-sum + degree norm) on 8 trn2 NeuronCores.

Sharding: destination nodes across cores (12500/core). Host buckets edges
by dst range, windows of 128 dst nodes, groups window edges by src table
chunk (32768 rows, int16 gather index limit), sorts groups by src for HBM
locality, pads each (window, chunk) group to a multiple of 128 with
idx=0 / dst=-1 sentinels, uniform K per (window, chunk) across cores
(single SPMD NEFF).

Launch 1 (per core): h2 = h * rsqrt(max(odc,1)) cast to bf16 (odc =
out-degree counts, host-precomputed integer bincount shipped as input;
the rsqrt + scale float math runs on device). Host concatenates slices
into the [100000, 256] bf16 gather table (col 128 = 1.0 constant).

Launch 2 (per core): per window: dma_gather (custom ucode) of 512B bf16
rows per chunk group, split into <=896-index sub-gathers cycled over 4
SWDGE queues (single-queue descriptor rings serialize at ~10.5 ns/desc;
4 queues + 896-idx instructions measure ~2.5 ns/desc); one-hot
[128, K, 128] via is_equal; K matmuls accumulate onehot^T @ row[0:129]
into PSUM [128, 129] (col 128 counts in-degree via the table's ones
column); out = agg * rsqrt(max(id,1)).
"""

import numpy as np
import ml_dtypes

import concourse.bass as bass
import concourse.bacc as bacc
import concourse.mybir as mybir
import concourse.tile as tile
from concourse.bass_utils import run_bass_kernel_spmd

N_SRC = 60000
N_DST = 40000
N_NODES = N_SRC + N_DST
D = 128
C = 8
P = 128
NPC = N_NODES // C
WN = 128
NW = (NPC + WN - 1) // WN
NPC_PAD = NW * WN
ELEM = 256               # bf16 row: h*od_r [0:128] | 1.0 | zeros
CHUNK = 32768
NCHUNK = (N_NODES + CHUNK - 1) // CHUNK
NQ = 4                   # SWDGE queues
MAXG = 7                 # max k-columns (128 idxs each) per gather inst

f32 = mybir.dt.float32
bf16 = mybir.dt.bfloat16
i16 = mybir.dt.int16
BF = ml_dtypes.bfloat16


# ---------------------------------------------------------------- host packing
def _pack_main(src_idx, dst_idx):
    """Group edges by (core, window, chunk); returns per-(w,c) K values and
    packed idx (int16 wrapped+replicated) and dstl (bf16, p-minor) arrays."""
    order = np.argsort(dst_idx, kind="stable")
    s_src = src_idx[order]
    s_dst = dst_idx[order]
    core_of = s_dst // NPC
    win_of = (s_dst - core_of * NPC) // WN
    chunk_of = s_src // CHUNK

    counts = np.zeros((C, NW, NCHUNK), dtype=np.int64)
    np.add.at(counts, (core_of, win_of, chunk_of), 1)
    # K per (window, chunk): uniform across cores; 0 => group skipped
    Kwc = np.zeros((NW, NCHUNK), dtype=np.int64)
    for w in range(NW):
        for ch in range(NCHUNK):
            m = counts[:, w, ch].max()
            Kwc[w, ch] = -(-m // P) if m > 0 else 0

    kcol0 = np.zeros((NW, NCHUNK), dtype=np.int64)  # dstl column offsets
    icol0 = np.zeros((NW, NCHUNK), dtype=np.int64)  # idx column offsets
    kacc = iacc = 0
    for w in range(NW):
        for ch in range(NCHUNK):
            kcol0[w, ch] = kacc
            icol0[w, ch] = iacc
            kacc += Kwc[w, ch]
            iacc += Kwc[w, ch] * 8

    idxs = np.zeros((C, P, iacc), dtype=np.int16)
    dstl = np.full((C, P, kacc), -1.0, dtype=np.float32)

    # edge ranges: sort order is (core, window) by dst; within that, chunk
    # grouping + src sort
    starts2 = np.concatenate(
        [[0], np.cumsum(counts.sum(axis=2).reshape(-1))]
    )
    for c in range(C):
        for w in range(NW):
            gi = c * NW + w
            lo, hi = starts2[gi], starts2[gi + 1]
            if hi == lo:
                continue
            g_src = s_src[lo:hi]
            g_dst = s_dst[lo:hi]
            so = np.argsort(g_src, kind="stable")  # chunk-major + src-sorted
            g_src = g_src[so]
            g_dst = g_dst[so]
            g_ch = g_src // CHUNK
            for ch in range(NCHUNK):
                sel = g_ch == ch
                n = int(sel.sum())
                if n == 0:
                    continue
                K = int(Kwc[w, ch])
                loc = np.zeros(K * P, dtype=np.int64)
                loc[:n] = g_src[sel] - ch * CHUNK
                dl = np.full(K * P, -1.0, dtype=np.float32)
                dl[:n] = (g_dst[sel] - (c * NPC + w * WN)).astype(np.float32)
                # idx j at wrapped [j%16, j//16], replicated to 8x16 rows
                wr = loc.astype(np.int16).reshape(K * 8, 16).T
                idxs[c, :, icol0[w, ch] : icol0[w, ch] + K * 8] = np.tile(
                    wr, (8, 1)
                )
                # dstl: edge j at [j%128, col kcol0 + j//128]
                dstl[c, np.arange(K * P) % P,
                     kcol0[w, ch] + np.arange(K * P) // P] = dl
    return Kwc, kcol0, icol0, idxs, dstl.astype(BF)


# ---------------------------------------------------------------- bass builders
def _build_degree_nc(repeat=1):
    """h2s = bf16(h * rsqrt(max(odc, 1))).

    Flat layout: partition p holds NW consecutive node rows (nodes
    [NW*p, NW*(p+1))); odc[p, j] is node NW*p+j's count. Two large DMAs
    instead of per-window transfers."""
    nc = bacc.Bacc("TRN2", target_bir_lowering=False)
    h_d = nc.dram_tensor("h_slice", [P, NW, D], f32, kind="ExternalInput")
    odc_d = nc.dram_tensor("odc", [P, NW], f32, kind="ExternalInput")
    h2_d = nc.dram_tensor("h2s", [P, NW, D], bf16, kind="ExternalOutput")

    with tile.TileContext(nc) as tc:
        with (
            tc.tile_pool(name="cst", bufs=1) as cst,
            tc.tile_pool(name="work", bufs=2) as wk,
        ):
            odc = cst.tile([P, NW], f32)
            nc.sync.dma_start(odc[:], odc_d[:])
            clamped = cst.tile([P, NW], f32)
            nc.vector.tensor_scalar_max(clamped[:], odc[:], 1.0)
            sq = cst.tile([P, NW], f32)
            nc.scalar.activation(
                sq[:], clamped[:], mybir.ActivationFunctionType.Sqrt
            )
            odr = cst.tile([P, NW], f32)
            nc.vector.reciprocal(odr[:], sq[:])

            def body(_=None):
                h_t = wk.tile([P, NW, D], f32, tag="h")
                nc.sync.dma_start(h_t[:], h_d[:])
                h2_t = wk.tile([P, NW, D], bf16, tag="h2")
                nc.vector.tensor_tensor(
                    out=h2_t[:],
                    in0=h_t[:],
                    in1=odr[:, :, None].to_broadcast([P, NW, D]),
                    op=mybir.AluOpType.mult,
                )
                nc.sync.dma_start(h2_d[:], h2_t[:])

            if repeat > 1:
                with tc.For_i(0, repeat, 1):
                    body()
            else:
                body()
    nc.compile()
    return nc


def _build_main_nc(Kwc, kcol0, icol0, repeat=1, parts='all'):
    nc = bacc.Bacc("TRN2", target_bir_lowering=False, num_swdge_queues=NQ)
    kcols = int(Kwc.sum())
    icols = kcols * 8
    h2_d = nc.dram_tensor("h2", [N_NODES, ELEM], bf16, kind="ExternalInput")
    idx_d = nc.dram_tensor("idxs", [P, icols], i16, kind="ExternalInput")
    dstl_d = nc.dram_tensor("dstl", [P, kcols], bf16, kind="ExternalInput")
    iota_d = nc.dram_tensor("iota", [P, WN], bf16, kind="ExternalInput")
    out_d = nc.dram_tensor("out_slice", [NPC_PAD, D], f32, kind="ExternalOutput")

    chunk_rows = [min(CHUNK, N_NODES - ch * CHUNK) for ch in range(NCHUNK)]
    qn = [0]  # round-robin queue counter

    with tile.TileContext(nc) as tc:
        with (
            tc.tile_pool(name="cst", bufs=1) as cst,
            tc.tile_pool(name="msgs", bufs=4) as mp,
            tc.tile_pool(name="work", bufs=3) as wk,
            tc.tile_pool(name="psum", bufs=4, space="PSUM") as ps,
        ):
            idxs = cst.tile([P, icols], i16)
            nc.sync.dma_start(idxs[:], idx_d[:])
            dstl = cst.tile([P, kcols], bf16)
            nc.sync.dma_start(dstl[:], dstl_d[:])
            iota = cst.tile([P, WN], bf16)
            nc.sync.dma_start(iota[:], iota_d[:])

            def body(_=None):
                for w in range(NW):
                    Ks = [int(Kwc[w, ch]) for ch in range(NCHUNK)]
                    Kw = sum(Ks)
                    kbase = int(kcol0[w, 0])
                    msgs = mp.tile([P, Kw, ELEM], bf16, tag="msgs")
                    kk = 0
                    for ch in range(NCHUNK):
                        K = Ks[ch]
                        if K == 0:
                            continue
                        ic0 = int(icol0[w, ch])
                        for k0 in range(0, K, MAXG):
                            Ksub = min(MAXG, K - k0)
                            nc.gpsimd.dma_gather(
                                out_ap=msgs[:, kk + k0 : kk + k0 + Ksub, :],
                                in_ap=h2_d[
                                    ch * CHUNK : ch * CHUNK + chunk_rows[ch], :
                                ],
                                idxs_ap=idxs[
                                    :, ic0 + k0 * 8 : ic0 + (k0 + Ksub) * 8
                                ],
                                num_idxs=Ksub * P,
                                num_idxs_reg=Ksub * P,
                                elem_size=ELEM,
                                queue_num=qn[0] % NQ,
                            )
                            qn[0] += 1
                        kk += K
                    if parts == "gather":
                        continue
                    oh = wk.tile([P, Kw, WN], bf16, tag="oh")
                    nc.vector.tensor_tensor(
                        out=oh[:],
                        in0=dstl[:, kbase : kbase + Kw, None].to_broadcast(
                            [P, Kw, WN]
                        ),
                        in1=iota[:, None, :].to_broadcast([P, Kw, WN]),
                        op=mybir.AluOpType.is_equal,
                    )
                    if parts == "onehot":
                        continue
                    acc = ps.tile([WN, D + 1], f32, space="PSUM")
                    for k in range(Kw):
                        nc.tensor.matmul(
                            acc[:],
                            lhsT=oh[:, k, :],
                            rhs=msgs[:, k, 0 : D + 1],
                            start=(k == 0),
                            stop=(k == Kw - 1),
                        )
                    clamped = wk.tile([WN, 1], f32, tag="cl")
                    nc.vector.tensor_scalar_max(
                        clamped[:], acc[:, D : D + 1], 1.0
                    )
                    sq = wk.tile([WN, 1], f32, tag="sq")
                    nc.scalar.activation(
                        sq[:], clamped[:], mybir.ActivationFunctionType.Sqrt
                    )
                    rsq = wk.tile([WN, 1], f32, tag="rsq")
                    nc.vector.reciprocal(rsq[:], sq[:])
                    fin = wk.tile([WN, D], f32, tag="fin")
                    nc.vector.tensor_scalar_mul(fin[:], acc[:, 0:D], rsq[:, 0:1])
                    nc.sync.dma_start(out_d[w * WN : (w + 1) * WN, :], fin[:])

            if repeat > 1:
                with tc.For_i(0, repeat, 1):
                    body()
            else:
                body()
    nc.compile()
    return nc


# ---------------------------------------------------------------- entry point
def kernel(src_embedding, dst_embedding, src_idx, dst_idx):
    src_embedding = np.asarray(src_embedding, dtype=np.float32)
    dst_embedding = np.asarray(dst_embedding, dtype=np.float32)
    src_idx = np.asarray(src_idx).astype(np.int64)
    dst_idx = np.asarray(dst_idx).astype(np.int64)

    iota_np = np.broadcast_to(
        np.arange(WN, dtype=np.float32), (P, WN)
    ).astype(BF)
    h_full = np.concatenate([src_embedding, dst_embedding], axis=0)

    # ---- launch 1: out-degree rsqrt + table scale on device
    odc_full = np.bincount(src_idx, minlength=N_NODES).astype(np.float32)
    nc1 = _build_degree_nc()
    in_maps1 = []
    for c in range(C):
        hs = np.zeros((NPC_PAD, D), dtype=np.float32)
        hs[:NPC] = h_full[c * NPC : (c + 1) * NPC]
        oc = np.zeros((NPC_PAD,), dtype=np.float32)
        oc[:NPC] = odc_full[c * NPC : (c + 1) * NPC]
        in_maps1.append(
            {
                "h_slice": np.ascontiguousarray(hs.reshape(P, NW, D)),
                "odc": np.ascontiguousarray(oc.reshape(P, NW)),
            }
        )
    res1 = run_bass_kernel_spmd(nc1, in_maps1, core_ids=list(range(C)))
    kernel.last_res1 = res1

    # ---- host glue: assemble bf16 gather table (layout only)
    h2 = np.zeros((N_NODES, ELEM), dtype=BF)
    for c in range(C):
        h2s = res1.results[c]["h2s"].reshape(NPC_PAD, D)
        h2[c * NPC : (c + 1) * NPC, :D] = h2s[:NPC]
    h2[:, D] = np.float32(1.0)

    # ---- launch 2: gather + aggregate + normalize
    Kwc, kcol0, icol0, idxs, dstl = _pack_main(src_idx, dst_idx)
    nc2 = _build_main_nc(Kwc, kcol0, icol0)
    in_maps2 = [
        {
            "h2": h2,
            "idxs": np.ascontiguousarray(idxs[c]),
            "dstl": np.ascontiguousarray(dstl[c]),
            "iota": iota_np,
        }
        for c in range(C)
    ]
    res2 = run_bass_kernel_spmd(nc2, in_maps2, core_ids=list(range(C)))
    kernel.last_res2 = res2
    out = np.concatenate(
        [res2.results[c]["out_slice"][:NPC] for c in range(C)], axis=0
    )
    return out
